# revision 16
# baseline (speedup 1.0000x reference)
"""Trainium2 Bass kernel for nn_MiddleLayerDecoder.

Computation (see problem):
  feats = relu MLP(x): 256 -> 256 -> 128 -> 64          [N, 64]
  rel   = (feats @ Wdec + bdec).reshape(N*K, 3)
  dec   = relu MLP over rows (n,k):
            concat([x_n, feats_n, rel_{n,k}]) -> 256 -> 256 -> 256
  cluster = repeat(arange(N), K)

Key transforms:
  * Pure data parallel over nodes: 8 cores x 6250 nodes.
  * First per-point layer is algebraically refactored so it runs at N rows,
    not N*K:  h1[n,k] = relu(x_n @ W0a + feats_n @ B_k + d_k)  with
    B_k = W0b + Wdec[:,3k:3k+3] @ W0c  and  d_k = b0 + bdec[3k:3k+3] @ W0c
    (host-precomputed).  u = x @ W0a is computed once per node tile and
    injected into each k's PSUM via an identity matmul.
  * Activations kept feature-major ([features, nodes]) on chip: matmul
    lhsT = weights (stationary), rhs = activations; per-partition bias.
    x is transposed on the host so no on-device transposes are needed.
  * The last layer flips to point-major (lhsT = activation tile) so the
    mandatory PSUM->SBUF relu pass writes the DMA-contiguous layout;
    its bias is injected as a K=1 matmul with a ones row.
  * PSUM accumulation chains serialize the PE array, so matmuls of
    different PSUM regions are interleaved to keep the pipeline deep.
  * cluster is input-independent: host numpy.
"""

import os
import sys

sys.path.insert(0, "/opt/trn_rl_repo")

import numpy as np

N_NODES = 50000
D_IN = 256
KPTS = 8
N_CORES = 8
R = N_NODES // N_CORES          # nodes per core
W = 1024                        # column (node) tile width
NTILES = (R + W - 1) // W       # 7 (6 full + 106 tail)

# compute dtype for matmul inputs: "float32" or "bfloat16"
CDT_NAME = os.environ.get("KERNEL_CDT", "bfloat16")

_cache = {}


def _ceil_div(a, b):
    return (a + b - 1) // b


def _build(cdt_name):
    from contextlib import ExitStack

    import concourse.mybir as mybir
    import concourse.tile as tile
    from concourse import bacc

    f32 = mybir.dt.float32
    cdt = getattr(mybir.dt, cdt_name)
    AF = mybir.ActivationFunctionType
    ALU = mybir.AluOpType

    nc = bacc.Bacc("TRN2", target_bir_lowering=False, debug=False,
                   enable_asserts=True, num_devices=N_CORES)

    def din(name, shape, dt):
        return nc.dram_tensor(name, shape, dt, kind="ExternalInput").ap()

    xT = din("xT", [D_IN, R], cdt)
    wg0 = din("wg0", [256, 256], cdt)
    wg1 = din("wg1", [256, 128], cdt)
    wg2 = din("wg2", [128, 64], cdt)
    bg0 = din("bg0", [128, 2], f32)
    bg1 = din("bg1", [128, 1], f32)
    bg2 = din("bg2", [64, 1], f32)
    wdec = din("wdec", [64, 24], cdt)
    bdec8 = din("bdec8", [1, 192], cdt)
    w0a = din("w0a", [256, 256], cdt)
    bmat = din("bmat", [65, 2048], cdt)
    dvec = din("dvec", [128, 16], f32)
    w1 = din("w1", [256, 256], cdt)
    b1 = din("b1", [128, 2], f32)
    w2 = din("w2", [256, 256], cdt)
    b2x2 = din("b2x2", [1, 512], cdt)
    ident = din("ident", [128, 128], cdt)
    ones = din("ones", [1, 128], cdt)

    rel_out = nc.dram_tensor("rel", [R * KPTS, 3], f32, kind="ExternalOutput").ap()
    dec_out = nc.dram_tensor("dec", [R * KPTS, 256], f32, kind="ExternalOutput").ap()

    # node-major views: [R, K*3] / [R, K*256]
    rel_v = rel_out.rearrange("(n e) c -> n (e c)", e=KPTS)
    dec_v = dec_out.rearrange("(n e) c -> n (e c)", e=KPTS)

    with tile.TileContext(nc) as tc, ExitStack() as ctx:
        wts = ctx.enter_context(tc.tile_pool(name="wts", bufs=1))
        xt_p = ctx.enter_context(tc.tile_pool(name="xt", bufs=3))
        gact_p = ctx.enter_context(tc.tile_pool(name="gact", bufs=3))
        f_p = ctx.enter_context(tc.tile_pool(name="feats", bufs=2))
        u_p = ctx.enter_context(tc.tile_pool(name="u", bufs=2))
        h1_p = ctx.enter_context(tc.tile_pool(name="h1", bufs=3))
        h2_p = ctx.enter_context(tc.tile_pool(name="h2", bufs=3))
        d_p = ctx.enter_context(tc.tile_pool(name="dst", bufs=4))
        rel_p = ctx.enter_context(tc.tile_pool(name="relst", bufs=2))
        ps_mm = ctx.enter_context(tc.tile_pool(name="psmm", bufs=4, space="PSUM"))
        ps_fl = ctx.enter_context(tc.tile_pool(name="psfl", bufs=4, space="PSUM"))

        def load_w(ap_dram, shape, dt, tag):
            t = wts.tile(shape, dt, tag=tag)
            nc.sync.dma_start(t[:, :], ap_dram)
            return t

        # --- weight preload (split 256-row tensors into two 128-part tiles)
        wg0_sb = [load_w(wg0[i * 128:(i + 1) * 128, :], [128, 256], cdt, f"wg0{i}") for i in range(2)]
        wg1_sb = [load_w(wg1[i * 128:(i + 1) * 128, :], [128, 128], cdt, f"wg1{i}") for i in range(2)]
        wg2_sb = load_w(wg2[:, :], [128, 64], cdt, "wg2")
        w0a_sb = [load_w(w0a[i * 128:(i + 1) * 128, :], [128, 256], cdt, f"w0a{i}") for i in range(2)]
        w1_sb = [load_w(w1[i * 128:(i + 1) * 128, :], [128, 256], cdt, f"w1{i}") for i in range(2)]
        w2_sb = [load_w(w2[i * 128:(i + 1) * 128, :], [128, 256], cdt, f"w2{i}") for i in range(2)]
        bmat_sb = load_w(bmat[:, :], [65, 2048], cdt, "bmat")
        wdec_sb = load_w(wdec[:, :], [64, 24], cdt, "wdec")
        bdec_sb = load_w(bdec8[:, :], [1, 192], cdt, "bdec8")
        b2_sb = load_w(b2x2[:, :], [1, 512], cdt, "b2x2")
        bg0_sb = load_w(bg0[:, :], [128, 2], f32, "bg0")
        bg1_sb = load_w(bg1[:, :], [128, 1], f32, "bg1")
        bg2_sb = load_w(bg2[:, :], [64, 1], f32, "bg2")
        dvec_sb = load_w(dvec[:, :], [128, 16], f32, "dvec")
        b1_sb = load_w(b1[:, :], [128, 2], f32, "b1")
        id_sb = load_w(ident[:, :], [128, 128], cdt, "ident")
        on_sb = load_w(ones[:, :], [1, 128], cdt, "ones")

        MMF = 512  # max matmul free size (fp32 PSUM bank)

        for t in range(NTILES):
            n0 = t * W
            w = min(W, R - n0)
            halves = [(ho, min(MMF, w - ho)) for ho in range(0, w, MMF)]
            nm = _ceil_div(w, 128)

            # ---- load xT tile ([256, w] as 2 partition chunks)
            xt_sb = []
            for c in range(2):
                xt = xt_p.tile([128, W], cdt, tag=f"xt{c}")
                nc.sync.dma_start(xt[:, :w], xT[c * 128:(c + 1) * 128, n0:n0 + w])
                xt_sb.append(xt)

            # ---- global MLP layer 0: 256 -> 256 (cin-outer: shared lhsT,
            #      adjacent matmuls hit different psum tiles)
            f0_sb = []
            for c2 in range(2):
                f0 = gact_p.tile([128, W], cdt, tag=f"f0{c2}")
                pss = [ps_mm.tile([128, MMF], f32, name=f"psg0_{t}_{c2}_{hi}",
                                  tag="psmm") for hi in range(len(halves))]
                for cin in range(2):
                    for hi, (ho, hw) in enumerate(halves):
                        nc.tensor.matmul(
                            pss[hi][:, :hw],
                            wg0_sb[cin][:, c2 * 128:(c2 + 1) * 128],
                            xt_sb[cin][:, ho:ho + hw],
                            start=(cin == 0), stop=(cin == 1))
                for hi, (ho, hw) in enumerate(halves):
                    nc.scalar.activation(f0[:, ho:ho + hw], pss[hi][:, :hw],
                                         AF.Relu, bias=bg0_sb[:, c2:c2 + 1])
                f0_sb.append(f0)

            # ---- layer 1: 256 -> 128
            f1 = gact_p.tile([128, W], cdt, tag="f1")
            pss = [ps_mm.tile([128, MMF], f32, name=f"psg1_{t}_{hi}", tag="psmm")
                   for hi in range(len(halves))]
            for cin in range(2):
                for hi, (ho, hw) in enumerate(halves):
                    nc.tensor.matmul(pss[hi][:, :hw], wg1_sb[cin],
                                     f0_sb[cin][:, ho:ho + hw],
                                     start=(cin == 0), stop=(cin == 1))
            for hi, (ho, hw) in enumerate(halves):
                nc.scalar.activation(f1[:, ho:ho + hw], pss[hi][:, :hw],
                                     AF.Relu, bias=bg1_sb[:, 0:1])

            # ---- layer 2: 128 -> 64  (feats)
            fT = f_p.tile([65, W], cdt, tag="fT")
            nc.gpsimd.memset(fT[64:65, :w], 1.0)
            for ho, hw in halves:
                ps = ps_mm.tile([128, MMF], f32, tag="psmm")
                nc.tensor.matmul(ps[:64, :hw], wg2_sb,
                                 f1[:, ho:ho + hw], start=True, stop=True)
                nc.scalar.activation(fT[:64, ho:ho + hw], ps[:64, :hw], AF.Relu,
                                     bias=bg2_sb[:64, 0:1])

            # ---- relative points: feats @ Wdec + bdec, point-major flip
            psr = ps_fl.tile([128, 512], f32, tag="psfl")
            nc.tensor.matmul(psr[:128, :nm * 24], on_sb[:1, :128],
                             bdec_sb[:1, :nm * 24], start=True, stop=False,
                             skip_group_check=True)
            for j in range(nm):
                mj = min(128, w - j * 128)
                nc.tensor.matmul(psr[:mj, j * 24:j * 24 + 24],
                                 fT[:64, j * 128:j * 128 + mj],
                                 wdec_sb[:64, :],
                                 start=False, stop=True, skip_group_check=True)
            relst = rel_p.tile([128, 192], f32, tag="relst")
            nc.vector.tensor_copy(relst[:, :nm * 24], psr[:, :nm * 24])
            if w == W:
                dram = rel_v[n0:n0 + w, :].rearrange("(j i) c -> i j c", i=128)
                nc.sync.dma_start(dram, relst[:, :].rearrange("i (j c) -> i j c", c=24))
            else:
                nc.sync.dma_start(rel_v[n0:n0 + w, :], relst[:w, :24])

            # ---- u = x @ W0a (shared across k), no activation
            u_sb = []
            for c2 in range(2):
                u = u_p.tile([128, W], cdt, tag=f"u{c2}")
                pss = [ps_mm.tile([128, MMF], f32, name=f"psu_{t}_{c2}_{hi}",
                                  tag="psmm") for hi in range(len(halves))]
                for cin in range(2):
                    for hi, (ho, hw) in enumerate(halves):
                        nc.tensor.matmul(
                            pss[hi][:, :hw],
                            w0a_sb[cin][:, c2 * 128:(c2 + 1) * 128],
                            xt_sb[cin][:, ho:ho + hw],
                            start=(cin == 0), stop=(cin == 1))
                for hi, (ho, hw) in enumerate(halves):
                    nc.vector.tensor_copy(u[:, ho:ho + hw], pss[hi][:, :hw])
                u_sb.append(u)

            # ---- per-k decode: h1 -> h2 -> h3 (flip) -> DMA
            for k in range(KPTS):
                # h1: inject u (identity matmul, 4 adjacent = shared lhsT),
                # then B_k accumulate, then fused DVE add-bias+relu
                h1_sb = [h1_p.tile([128, W], cdt, name=f"h1_{t}_{k}_{c2}",
                                   tag=f"h1{c2}") for c2 in range(2)]
                h1pre = [h1_p.tile([128, W], cdt, name=f"h1p_{t}_{k}_{c2}",
                                   tag=f"h1p{c2}") for c2 in range(2)]
                h1ps = {}
                for c2 in range(2):
                    for hi, (ho, hw) in enumerate(halves):
                        ps = ps_mm.tile([128, MMF], f32,
                                        name=f"psh1_{t}_{k}_{c2}_{hi}", tag="psmm")
                        h1ps[c2, hi] = ps
                        nc.tensor.matmul(
                            ps[:, :hw],
                            bmat_sb[:65, k * 256 + c2 * 128:k * 256 + (c2 + 1) * 128],
                            fT[:65, ho:ho + hw],
                            start=True, stop=True)
                for c2 in range(2):
                    for hi, (ho, hw) in enumerate(halves):
                        nc.vector.tensor_tensor(
                            out=h1pre[c2][:, ho:ho + hw], in0=h1ps[c2, hi][:, :hw],
                            in1=u_sb[c2][:, ho:ho + hw], op=ALU.add)
                        nc.vector.tensor_scalar_max(
                            out=h1_sb[c2][:, ho:ho + hw],
                            in0=h1pre[c2][:, ho:ho + hw], scalar1=0.0)

                # h2: 256 -> 256, cin-outer
                h2_sb = []
                for c2 in range(2):
                    h2 = h2_p.tile([128, W], cdt, tag=f"h2{c2}")
                    pss = [ps_mm.tile([128, MMF], f32,
                                      name=f"psh2_{t}_{k}_{c2}_{hi}", tag="psmm")
                           for hi in range(len(halves))]
                    for cin in range(2):
                        for hi, (ho, hw) in enumerate(halves):
                            nc.tensor.matmul(
                                pss[hi][:, :hw],
                                w1_sb[cin][:, c2 * 128:(c2 + 1) * 128],
                                h1_sb[cin][:, ho:ho + hw],
                                start=(cin == 0), stop=(cin == 1))
                    for hi, (ho, hw) in enumerate(halves):
                        nc.scalar.activation(h2[:, ho:ho + hw], pss[hi][:, :hw],
                                             AF.Relu, bias=b1_sb[:, c2:c2 + 1])
                    h2_sb.append(h2)

                # h3: point-major flip; pair-groups of 2 psum tiles with
                # matmuls interleaved across the 4 chunk-regions so no two
                # adjacent matmuls accumulate into the same region.
                dst = d_p.tile([128, 2048], f32, tag="dst")
                npairs = _ceil_div(nm, 2)
                for g in range(0, npairs, 2):
                    pairs = [p for p in (g, g + 1) if p < npairs]
                    pfs, meta = [], []
                    for p in pairs:
                        pf = ps_fl.tile([128, 512], f32,
                                        name=f"pf_{t}_{k}_{p}", tag="psfl")
                        chunks = [j for j in (p * 2, p * 2 + 1) if j < nm]
                        mjs = [min(128, w - j * 128) for j in chunks]
                        pw = 256 * len(chunks)
                        nc.tensor.matmul(pf[:max(mjs), :pw], on_sb[:1, :max(mjs)],
                                         b2_sb[:1, :pw], start=True, stop=False,
                                         skip_group_check=True)
                        pfs.append(pf)
                        meta.append((chunks, mjs, pw))
                    for cin in range(2):
                        for jj in range(2):
                            for pi in range(len(pairs)):
                                chunks, mjs, pw = meta[pi]
                                if jj >= len(chunks):
                                    continue
                                j, mj = chunks[jj], mjs[jj]
                                off = jj * 256
                                nc.tensor.matmul(
                                    pfs[pi][:mj, off:off + 256],
                                    h2_sb[cin][:, j * 128:j * 128 + mj],
                                    w2_sb[cin], start=False, stop=(cin == 1),
                                    skip_group_check=True)
                    for pi, p in enumerate(pairs):
                        chunks, mjs, pw = meta[pi]
                        o = dst[:, p * 512:p * 512 + pw]
                        if p % 2 == 0:
                            nc.scalar.activation(o, pfs[pi][:, :pw], AF.Relu,
                                                 bias=0.0)
                        else:
                            nc.vector.tensor_scalar_max(out=o, in0=pfs[pi][:, :pw],
                                                        scalar1=0.0)
                if w == W:
                    dram = dec_v[n0:n0 + w, k * 256:(k + 1) * 256].rearrange(
                        "(j i) c -> i j c", i=128)
                    nc.sync.dma_start(
                        dram, dst[:, :].rearrange("i (j c) -> i j c", c=256))
                else:
                    nc.sync.dma_start(dec_v[n0:n0 + w, k * 256:(k + 1) * 256],
                                      dst[:w, :256])

    nc.compile()
    return nc


def _get_nc(cdt_name):
    if cdt_name not in _cache:
        _cache[cdt_name] = _build(cdt_name)
    return _cache[cdt_name]


def _prep_inputs(inputs, cdt_name):
    import ml_dtypes

    np_cdt = np.float32 if cdt_name == "float32" else ml_dtypes.bfloat16

    def c(a):
        return np.ascontiguousarray(a, dtype=np_cdt)

    x = np.asarray(inputs["input_features"], dtype=np.float32)
    W0 = np.asarray(inputs["W0"], dtype=np.float32)
    W0a, W0b, W0c = W0[:256], W0[256:320], W0[320:323]
    Wdec = np.asarray(inputs["Wdec"], dtype=np.float32)
    bdec = np.asarray(inputs["bdec"], dtype=np.float32)
    b0 = np.asarray(inputs["b0"], dtype=np.float32)

    B = np.concatenate(
        [np.vstack([W0b + Wdec[:, 3 * k:3 * k + 3] @ W0c,
                    (b0 + bdec[3 * k:3 * k + 3] @ W0c)[None, :]])
         for k in range(KPTS)], axis=1)
    D = np.stack([b0 + bdec[3 * k:3 * k + 3] @ W0c for k in range(KPTS)])

    def pcol(v):  # [n*128] -> [128, n] per-partition bias columns
        return np.ascontiguousarray(v.reshape(-1, 128).T, dtype=np.float32)

    shared = {
        "wg0": c(inputs["Wg0"]), "wg1": c(inputs["Wg1"]), "wg2": c(inputs["Wg2"]),
        "bg0": pcol(np.asarray(inputs["bg0"], np.float32)),
        "bg1": pcol(np.asarray(inputs["bg1"], np.float32)),
        "bg2": np.asarray(inputs["bg2"], np.float32).reshape(64, 1).copy(),
        "wdec": c(Wdec),
        "bdec8": c(np.tile(bdec, KPTS)[None, :]),
        "w0a": c(W0a),
        "bmat": c(B),
        "dvec": np.ascontiguousarray(
            D.reshape(KPTS, 2, 128).transpose(2, 0, 1).reshape(128, KPTS * 2),
            dtype=np.float32),
        "w1": c(inputs["W1"]), "b1": pcol(np.asarray(inputs["b1"], np.float32)),
        "w2": c(inputs["W2"]),
        "b2x2": c(np.tile(np.asarray(inputs["b2"], np.float32), 2)[None, :]),
        "ident": c(np.eye(128, dtype=np.float32)),
        "ones": c(np.ones((1, 128), dtype=np.float32)),
    }
    in_maps = []
    for cid in range(N_CORES):
        m = dict(shared)
        m["xT"] = c(x[cid * R:(cid + 1) * R].T)
        in_maps.append(m)
    return in_maps


def _install_ntff_hook_module():
    """Make `antenv.axon_hooks` importable (image ships without it)."""
    import importlib.util

    try:
        import antenv.axon_hooks  # noqa: F401
        return
    except ImportError:
        pass
    path = "/opt/trn_rl_repo/antenv/axon_hooks.py"
    if not os.path.exists(path):
        return
    spec = importlib.util.spec_from_file_location("antenv.axon_hooks", path)
    mod = importlib.util.module_from_spec(spec)
    spec.loader.exec_module(mod)
    sys.modules["antenv.axon_hooks"] = mod
    import antenv

    antenv.axon_hooks = mod


def _maybe_enable_ldw_opt():
    """Optionally flip walrus --enable-ldw-opt via a driver shim
    (KERNEL_LDW_OPT=1). Correctness is validated against the reference."""
    if os.environ.get("KERNEL_LDW_OPT") != "1":
        return
    from concourse import bass_utils as bu

    if getattr(bu, "_ldw_shim", None):
        return
    real = bu.get_walrus_driver()
    shim = "/tmp/walrus_ldw_shim.sh"
    with open(shim, "w") as f:
        f.write("#!/bin/sh\n")
        f.write('args=""\n')
        f.write('for a in "$@"; do\n')
        f.write('  case "$a" in --enable-ldw-opt=false) a="--enable-ldw-opt=true";; esac\n')
        f.write('  args="$args \"$a\""\n')
        f.write("done\n")
        f.write(f'eval exec "{real}" $args\n')
    os.chmod(shim, 0o755)
    bu.get_walrus_driver = lambda: shim
    bu._ldw_shim = True


def kernel(**inputs):
    from concourse.bass_utils import run_bass_kernel_spmd

    _maybe_enable_ldw_opt()
    nc = _get_nc(CDT_NAME)
    in_maps = _prep_inputs(inputs, CDT_NAME)

    trace = os.environ.get("BASS_KERNEL_TRACE") == "1"
    if trace:
        _install_ntff_hook_module()
    res = run_bass_kernel_spmd(nc, in_maps, core_ids=list(range(N_CORES)),
                               trace=trace)
    global LAST_RESULT
    LAST_RESULT = res

    rel = np.concatenate([res.results[c]["rel"] for c in range(N_CORES)], axis=0)
    dec = np.concatenate([res.results[c]["dec"] for c in range(N_CORES)], axis=0)
    cluster = np.repeat(np.arange(N_NODES, dtype=np.int32), KPTS)
    return rel, dec, cluster


# revision 17
# speedup vs baseline: 1.0746x; 1.0746x over previous
"""Trainium2 Bass kernel for nn_MiddleLayerDecoder.

Computation (see problem):
  feats = relu MLP(x): 256 -> 256 -> 128 -> 64          [N, 64]
  rel   = (feats @ Wdec + bdec).reshape(N*K, 3)
  dec   = relu MLP over rows (n,k):
            concat([x_n, feats_n, rel_{n,k}]) -> 256 -> 256 -> 256
  cluster = repeat(arange(N), K)

Key transforms:
  * Pure data parallel over nodes: 8 cores x 6250 nodes.
  * First per-point layer is algebraically refactored so it runs at N rows,
    not N*K:  h1[n,k] = relu(x_n @ W0a + feats_n @ B_k + d_k)  with
    B_k = W0b + Wdec[:,3k:3k+3] @ W0c  and  d_k = b0 + bdec[3k:3k+3] @ W0c
    (host-precomputed).  u = x @ W0a is computed once per node tile and
    injected into each k's PSUM via an identity matmul.
  * Activations kept feature-major ([features, nodes]) on chip: matmul
    lhsT = weights (stationary), rhs = activations; per-partition bias.
    x is transposed on the host so no on-device transposes are needed.
  * The last layer flips to point-major (lhsT = activation tile) so the
    mandatory PSUM->SBUF relu pass writes the DMA-contiguous layout;
    its bias is injected as a K=1 matmul with a ones row.
  * PSUM accumulation chains serialize the PE array, so matmuls of
    different PSUM regions are interleaved to keep the pipeline deep.
  * cluster is input-independent: host numpy.
"""

import os
import sys

sys.path.insert(0, "/opt/trn_rl_repo")

import numpy as np

N_NODES = 50000
D_IN = 256
KPTS = 8
N_CORES = 8
R = N_NODES // N_CORES          # nodes per core
W = 1024                        # column (node) tile width
NTILES = (R + W - 1) // W       # 7 (6 full + 106 tail)

# compute dtype for matmul inputs: "float32" or "bfloat16"
CDT_NAME = os.environ.get("KERNEL_CDT", "bfloat16")

_cache = {}


def _ceil_div(a, b):
    return (a + b - 1) // b


def _build(cdt_name):
    from contextlib import ExitStack

    import concourse.mybir as mybir
    import concourse.tile as tile
    from concourse import bacc

    f32 = mybir.dt.float32
    cdt = getattr(mybir.dt, cdt_name)
    AF = mybir.ActivationFunctionType
    ALU = mybir.AluOpType

    nc = bacc.Bacc("TRN2", target_bir_lowering=False, debug=False,
                   enable_asserts=True, num_devices=N_CORES)

    def din(name, shape, dt):
        return nc.dram_tensor(name, shape, dt, kind="ExternalInput").ap()

    xT = din("xT", [D_IN, R], cdt)
    wg0 = din("wg0", [256, 256], cdt)
    wg1 = din("wg1", [256, 128], cdt)
    wg2 = din("wg2", [128, 64], cdt)
    bg0 = din("bg0", [128, 2], f32)
    bg1 = din("bg1", [128, 1], f32)
    bg2 = din("bg2", [64, 1], f32)
    wdec = din("wdec", [64, 24], cdt)
    bdec8 = din("bdec8", [1, 192], cdt)
    w0a = din("w0a", [256, 256], cdt)
    bmat = din("bmat", [65, 2048], cdt)
    dvec = din("dvec", [128, 16], f32)
    w1 = din("w1", [256, 256], cdt)
    b1 = din("b1", [128, 2], f32)
    w2 = din("w2", [256, 256], cdt)
    b2x2 = din("b2x2", [1, 512], cdt)
    ident = din("ident", [128, 128], cdt)
    ones = din("ones", [1, 128], cdt)

    rel_out = nc.dram_tensor("rel", [R * KPTS, 3], f32, kind="ExternalOutput").ap()
    dec_out = nc.dram_tensor("dec", [R * KPTS, 256], f32, kind="ExternalOutput").ap()

    # node-major views: [R, K*3] / [R, K*256]
    rel_v = rel_out.rearrange("(n e) c -> n (e c)", e=KPTS)
    dec_v = dec_out.rearrange("(n e) c -> n (e c)", e=KPTS)

    with tile.TileContext(nc) as tc, ExitStack() as ctx:
        wts = ctx.enter_context(tc.tile_pool(name="wts", bufs=1))
        xt_p = ctx.enter_context(tc.tile_pool(name="xt", bufs=3))
        gact_p = ctx.enter_context(tc.tile_pool(name="gact", bufs=3))
        f_p = ctx.enter_context(tc.tile_pool(name="feats", bufs=2))
        u_p = ctx.enter_context(tc.tile_pool(name="u", bufs=2))
        h1_p = ctx.enter_context(tc.tile_pool(name="h1", bufs=3))
        h2_p = ctx.enter_context(tc.tile_pool(name="h2", bufs=3))
        d_p = ctx.enter_context(tc.tile_pool(name="dst", bufs=4))
        rel_p = ctx.enter_context(tc.tile_pool(name="relst", bufs=2))
        ps_mm = ctx.enter_context(tc.tile_pool(name="psmm", bufs=6, space="PSUM"))
        ps_fl = ctx.enter_context(tc.tile_pool(name="psfl", bufs=2, space="PSUM"))

        def load_w(ap_dram, shape, dt, tag):
            t = wts.tile(shape, dt, tag=tag)
            nc.sync.dma_start(t[:, :], ap_dram)
            return t

        # --- weight preload (split 256-row tensors into two 128-part tiles)
        wg0_sb = [load_w(wg0[i * 128:(i + 1) * 128, :], [128, 256], cdt, f"wg0{i}") for i in range(2)]
        wg1_sb = [load_w(wg1[i * 128:(i + 1) * 128, :], [128, 128], cdt, f"wg1{i}") for i in range(2)]
        wg2_sb = load_w(wg2[:, :], [128, 64], cdt, "wg2")
        w0a_sb = [load_w(w0a[i * 128:(i + 1) * 128, :], [128, 256], cdt, f"w0a{i}") for i in range(2)]
        w1_sb = [load_w(w1[i * 128:(i + 1) * 128, :], [128, 256], cdt, f"w1{i}") for i in range(2)]
        w2_sb = [load_w(w2[i * 128:(i + 1) * 128, :], [128, 256], cdt, f"w2{i}") for i in range(2)]
        bmat_sb = load_w(bmat[:, :], [65, 2048], cdt, "bmat")
        wdec_sb = load_w(wdec[:, :], [64, 24], cdt, "wdec")
        bdec_sb = load_w(bdec8[:, :], [1, 192], cdt, "bdec8")
        b2_sb = load_w(b2x2[:, :], [1, 512], cdt, "b2x2")
        bg0_sb = load_w(bg0[:, :], [128, 2], f32, "bg0")
        bg1_sb = load_w(bg1[:, :], [128, 1], f32, "bg1")
        bg2_sb = load_w(bg2[:, :], [64, 1], f32, "bg2")
        dvec_sb = load_w(dvec[:, :], [128, 16], f32, "dvec")
        b1_sb = load_w(b1[:, :], [128, 2], f32, "b1")
        id_sb = load_w(ident[:, :], [128, 128], cdt, "ident")
        on_sb = load_w(ones[:, :], [1, 128], cdt, "ones")

        MMF = 512  # max matmul free size (fp32 PSUM bank)

        for t in range(NTILES):
            n0 = t * W
            w = min(W, R - n0)
            halves = [(ho, min(MMF, w - ho)) for ho in range(0, w, MMF)]
            nm = _ceil_div(w, 128)

            # ---- load xT tile ([256, w] as 2 partition chunks)
            xt_sb = []
            for c in range(2):
                xt = xt_p.tile([128, W], cdt, tag=f"xt{c}")
                nc.sync.dma_start(xt[:, :w], xT[c * 128:(c + 1) * 128, n0:n0 + w])
                xt_sb.append(xt)

            # ---- global MLP layer 0: 256 -> 256 (cin-outer: shared lhsT,
            #      adjacent matmuls hit different psum tiles)
            f0_sb = []
            for c2 in range(2):
                f0 = gact_p.tile([128, W], cdt, tag=f"f0{c2}")
                pss = [ps_mm.tile([128, MMF], f32, name=f"psg0_{t}_{c2}_{hi}",
                                  tag="psmm") for hi in range(len(halves))]
                for cin in range(2):
                    for hi, (ho, hw) in enumerate(halves):
                        nc.tensor.matmul(
                            pss[hi][:, :hw],
                            wg0_sb[cin][:, c2 * 128:(c2 + 1) * 128],
                            xt_sb[cin][:, ho:ho + hw],
                            start=(cin == 0), stop=(cin == 1))
                for hi, (ho, hw) in enumerate(halves):
                    nc.scalar.activation(f0[:, ho:ho + hw], pss[hi][:, :hw],
                                         AF.Relu, bias=bg0_sb[:, c2:c2 + 1])
                f0_sb.append(f0)

            # ---- layer 1: 256 -> 128
            f1 = gact_p.tile([128, W], cdt, tag="f1")
            pss = [ps_mm.tile([128, MMF], f32, name=f"psg1_{t}_{hi}", tag="psmm")
                   for hi in range(len(halves))]
            for cin in range(2):
                for hi, (ho, hw) in enumerate(halves):
                    nc.tensor.matmul(pss[hi][:, :hw], wg1_sb[cin],
                                     f0_sb[cin][:, ho:ho + hw],
                                     start=(cin == 0), stop=(cin == 1))
            for hi, (ho, hw) in enumerate(halves):
                nc.scalar.activation(f1[:, ho:ho + hw], pss[hi][:, :hw],
                                     AF.Relu, bias=bg1_sb[:, 0:1])

            # ---- layer 2: 128 -> 64  (feats)
            fT = f_p.tile([65, W], cdt, tag="fT")
            nc.gpsimd.memset(fT[64:65, :w], 1.0)
            for ho, hw in halves:
                ps = ps_mm.tile([128, MMF], f32, tag="psmm")
                nc.tensor.matmul(ps[:64, :hw], wg2_sb,
                                 f1[:, ho:ho + hw], start=True, stop=True)
                nc.scalar.activation(fT[:64, ho:ho + hw], ps[:64, :hw], AF.Relu,
                                     bias=bg2_sb[:64, 0:1])

            # ---- relative points: feats @ Wdec + bdec, point-major flip
            psr = ps_fl.tile([128, 512], f32, tag="psfl")
            nc.tensor.matmul(psr[:128, :nm * 24], on_sb[:1, :128],
                             bdec_sb[:1, :nm * 24], start=True, stop=False,
                             skip_group_check=True)
            for j in range(nm):
                mj = min(128, w - j * 128)
                nc.tensor.matmul(psr[:mj, j * 24:j * 24 + 24],
                                 fT[:64, j * 128:j * 128 + mj],
                                 wdec_sb[:64, :],
                                 start=False, stop=True, skip_group_check=True)
            relst = rel_p.tile([128, 192], f32, tag="relst")
            nc.vector.tensor_copy(relst[:, :nm * 24], psr[:, :nm * 24])
            if w == W:
                dram = rel_v[n0:n0 + w, :].rearrange("(j i) c -> i j c", i=128)
                nc.sync.dma_start(dram, relst[:, :].rearrange("i (j c) -> i j c", c=24))
            else:
                nc.sync.dma_start(rel_v[n0:n0 + w, :], relst[:w, :24])

            # ---- u = x @ W0a (shared across k), no activation
            u_sb = []
            for c2 in range(2):
                u = u_p.tile([128, W], cdt, tag=f"u{c2}")
                pss = [ps_mm.tile([128, MMF], f32, name=f"psu_{t}_{c2}_{hi}",
                                  tag="psmm") for hi in range(len(halves))]
                for cin in range(2):
                    for hi, (ho, hw) in enumerate(halves):
                        nc.tensor.matmul(
                            pss[hi][:, :hw],
                            w0a_sb[cin][:, c2 * 128:(c2 + 1) * 128],
                            xt_sb[cin][:, ho:ho + hw],
                            start=(cin == 0), stop=(cin == 1))
                for hi, (ho, hw) in enumerate(halves):
                    nc.vector.tensor_copy(u[:, ho:ho + hw], pss[hi][:, :hw])
                u_sb.append(u)

            # ---- per-k decode: h1 -> h2 -> h3 (flip) -> DMA
            for k in range(KPTS):
                # h1: inject u (identity matmul, 4 adjacent = shared lhsT),
                # then B_k accumulate, then fused DVE add-bias+relu
                h1_sb = [h1_p.tile([128, W], cdt, name=f"h1_{t}_{k}_{c2}",
                                   tag=f"h1{c2}") for c2 in range(2)]
                h1pre = [h1_p.tile([128, W], cdt, name=f"h1p_{t}_{k}_{c2}",
                                   tag=f"h1p{c2}") for c2 in range(2)]
                h1ps = {}
                for c2 in range(2):
                    for hi, (ho, hw) in enumerate(halves):
                        ps = ps_mm.tile([128, MMF], f32,
                                        name=f"psh1_{t}_{k}_{c2}_{hi}", tag="psmm")
                        h1ps[c2, hi] = ps
                        nc.tensor.matmul(
                            ps[:, :hw],
                            bmat_sb[:65, k * 256 + c2 * 128:k * 256 + (c2 + 1) * 128],
                            fT[:65, ho:ho + hw],
                            start=True, stop=True)
                for c2 in range(2):
                    for hi, (ho, hw) in enumerate(halves):
                        nc.vector.tensor_tensor(
                            out=h1pre[c2][:, ho:ho + hw], in0=h1ps[c2, hi][:, :hw],
                            in1=u_sb[c2][:, ho:ho + hw], op=ALU.add)
                        nc.vector.tensor_scalar_max(
                            out=h1_sb[c2][:, ho:ho + hw],
                            in0=h1pre[c2][:, ho:ho + hw], scalar1=0.0)

                # h2: 256 -> 256, cin-outer
                h2_sb = []
                for c2 in range(2):
                    h2 = h2_p.tile([128, W], cdt, tag=f"h2{c2}")
                    pss = [ps_mm.tile([128, MMF], f32,
                                      name=f"psh2_{t}_{k}_{c2}_{hi}", tag="psmm")
                           for hi in range(len(halves))]
                    for cin in range(2):
                        for hi, (ho, hw) in enumerate(halves):
                            nc.tensor.matmul(
                                pss[hi][:, :hw],
                                w1_sb[cin][:, c2 * 128:(c2 + 1) * 128],
                                h1_sb[cin][:, ho:ho + hw],
                                start=(cin == 0), stop=(cin == 1))
                    for hi, (ho, hw) in enumerate(halves):
                        nc.scalar.activation(h2[:, ho:ho + hw], pss[hi][:, :hw],
                                             AF.Relu, bias=b1_sb[:, c2:c2 + 1])
                    h2_sb.append(h2)

                # h3: point-major flip; pair-groups of 2 psum tiles with
                # matmuls interleaved across the 4 chunk-regions so no two
                # adjacent matmuls accumulate into the same region.
                dst = d_p.tile([128, 2048], f32, tag="dst")
                npairs = _ceil_div(nm, 2)
                for g in range(0, npairs, 2):
                    pairs = [p for p in (g, g + 1) if p < npairs]
                    pfs, meta = [], []
                    for p in pairs:
                        pf = ps_fl.tile([128, 512], f32,
                                        name=f"pf_{t}_{k}_{p}", tag="psfl")
                        chunks = [j for j in (p * 2, p * 2 + 1) if j < nm]
                        mjs = [min(128, w - j * 128) for j in chunks]
                        pw = 256 * len(chunks)
                        nc.tensor.matmul(pf[:max(mjs), :pw], on_sb[:1, :max(mjs)],
                                         b2_sb[:1, :pw], start=True, stop=False,
                                         skip_group_check=True)
                        pfs.append(pf)
                        meta.append((chunks, mjs, pw))
                    for cin in range(2):
                        for jj in range(2):
                            for pi in range(len(pairs)):
                                chunks, mjs, pw = meta[pi]
                                if jj >= len(chunks):
                                    continue
                                j, mj = chunks[jj], mjs[jj]
                                off = jj * 256
                                nc.tensor.matmul(
                                    pfs[pi][:mj, off:off + 256],
                                    h2_sb[cin][:, j * 128:j * 128 + mj],
                                    w2_sb[cin], start=False, stop=(cin == 1),
                                    skip_group_check=True)
                    for pi, p in enumerate(pairs):
                        chunks, mjs, pw = meta[pi]
                        o = dst[:, p * 512:p * 512 + pw]
                        if p % 2 == 0:
                            nc.scalar.activation(o, pfs[pi][:, :pw], AF.Relu,
                                                 bias=0.0)
                        else:
                            nc.vector.tensor_scalar_max(out=o, in0=pfs[pi][:, :pw],
                                                        scalar1=0.0)
                if w == W:
                    dram = dec_v[n0:n0 + w, k * 256:(k + 1) * 256].rearrange(
                        "(j i) c -> i j c", i=128)
                    nc.sync.dma_start(
                        dram, dst[:, :].rearrange("i (j c) -> i j c", c=256))
                else:
                    nc.sync.dma_start(dec_v[n0:n0 + w, k * 256:(k + 1) * 256],
                                      dst[:w, :256])

    nc.compile()
    return nc


def _get_nc(cdt_name):
    if cdt_name not in _cache:
        _cache[cdt_name] = _build(cdt_name)
    return _cache[cdt_name]


def _prep_inputs(inputs, cdt_name):
    import ml_dtypes

    np_cdt = np.float32 if cdt_name == "float32" else ml_dtypes.bfloat16

    def c(a):
        return np.ascontiguousarray(a, dtype=np_cdt)

    x = np.asarray(inputs["input_features"], dtype=np.float32)
    W0 = np.asarray(inputs["W0"], dtype=np.float32)
    W0a, W0b, W0c = W0[:256], W0[256:320], W0[320:323]
    Wdec = np.asarray(inputs["Wdec"], dtype=np.float32)
    bdec = np.asarray(inputs["bdec"], dtype=np.float32)
    b0 = np.asarray(inputs["b0"], dtype=np.float32)

    B = np.concatenate(
        [np.vstack([W0b + Wdec[:, 3 * k:3 * k + 3] @ W0c,
                    (b0 + bdec[3 * k:3 * k + 3] @ W0c)[None, :]])
         for k in range(KPTS)], axis=1)
    D = np.stack([b0 + bdec[3 * k:3 * k + 3] @ W0c for k in range(KPTS)])

    def pcol(v):  # [n*128] -> [128, n] per-partition bias columns
        return np.ascontiguousarray(v.reshape(-1, 128).T, dtype=np.float32)

    shared = {
        "wg0": c(inputs["Wg0"]), "wg1": c(inputs["Wg1"]), "wg2": c(inputs["Wg2"]),
        "bg0": pcol(np.asarray(inputs["bg0"], np.float32)),
        "bg1": pcol(np.asarray(inputs["bg1"], np.float32)),
        "bg2": np.asarray(inputs["bg2"], np.float32).reshape(64, 1).copy(),
        "wdec": c(Wdec),
        "bdec8": c(np.tile(bdec, KPTS)[None, :]),
        "w0a": c(W0a),
        "bmat": c(B),
        "dvec": np.ascontiguousarray(
            D.reshape(KPTS, 2, 128).transpose(2, 0, 1).reshape(128, KPTS * 2),
            dtype=np.float32),
        "w1": c(inputs["W1"]), "b1": pcol(np.asarray(inputs["b1"], np.float32)),
        "w2": c(inputs["W2"]),
        "b2x2": c(np.tile(np.asarray(inputs["b2"], np.float32), 2)[None, :]),
        "ident": c(np.eye(128, dtype=np.float32)),
        "ones": c(np.ones((1, 128), dtype=np.float32)),
    }
    in_maps = []
    for cid in range(N_CORES):
        m = dict(shared)
        m["xT"] = c(x[cid * R:(cid + 1) * R].T)
        in_maps.append(m)
    return in_maps


def _install_ntff_hook_module():
    """Make `antenv.axon_hooks` importable (image ships without it)."""
    import importlib.util

    try:
        import antenv.axon_hooks  # noqa: F401
        return
    except ImportError:
        pass
    path = "/opt/trn_rl_repo/antenv/axon_hooks.py"
    if not os.path.exists(path):
        return
    spec = importlib.util.spec_from_file_location("antenv.axon_hooks", path)
    mod = importlib.util.module_from_spec(spec)
    spec.loader.exec_module(mod)
    sys.modules["antenv.axon_hooks"] = mod
    import antenv

    antenv.axon_hooks = mod


def _maybe_enable_ldw_opt():
    """Optionally flip walrus --enable-ldw-opt via a driver shim
    (KERNEL_LDW_OPT=1). Correctness is validated against the reference."""
    if os.environ.get("KERNEL_LDW_OPT") != "1":
        return
    from concourse import bass_utils as bu

    if getattr(bu, "_ldw_shim", None):
        return
    real = bu.get_walrus_driver()
    shim = "/tmp/walrus_ldw_shim.sh"
    with open(shim, "w") as f:
        f.write("#!/bin/sh\n")
        f.write('args=""\n')
        f.write('for a in "$@"; do\n')
        f.write('  case "$a" in --enable-ldw-opt=false) a="--enable-ldw-opt=true";; esac\n')
        f.write('  args="$args \"$a\""\n')
        f.write("done\n")
        f.write(f'eval exec "{real}" $args\n')
    os.chmod(shim, 0o755)
    bu.get_walrus_driver = lambda: shim
    bu._ldw_shim = True


def kernel(**inputs):
    from concourse.bass_utils import run_bass_kernel_spmd

    _maybe_enable_ldw_opt()
    nc = _get_nc(CDT_NAME)
    in_maps = _prep_inputs(inputs, CDT_NAME)

    trace = os.environ.get("BASS_KERNEL_TRACE") == "1"
    if trace:
        _install_ntff_hook_module()
    res = run_bass_kernel_spmd(nc, in_maps, core_ids=list(range(N_CORES)),
                               trace=trace)
    global LAST_RESULT
    LAST_RESULT = res

    rel = np.concatenate([res.results[c]["rel"] for c in range(N_CORES)], axis=0)
    dec = np.concatenate([res.results[c]["dec"] for c in range(N_CORES)], axis=0)
    cluster = np.repeat(np.arange(N_NODES, dtype=np.int32), KPTS)
    return rel, dec, cluster


# revision 18
# speedup vs baseline: 1.1083x; 1.0314x over previous
"""Trainium2 Bass kernel for nn_MiddleLayerDecoder.

Computation (see problem):
  feats = relu MLP(x): 256 -> 256 -> 128 -> 64          [N, 64]
  rel   = (feats @ Wdec + bdec).reshape(N*K, 3)
  dec   = relu MLP over rows (n,k):
            concat([x_n, feats_n, rel_{n,k}]) -> 256 -> 256 -> 256
  cluster = repeat(arange(N), K)

Key transforms:
  * Pure data parallel over nodes: 8 cores x 6250 nodes.
  * First per-point layer is algebraically refactored so it runs at N rows,
    not N*K:  h1[n,k] = relu(x_n @ W0a + feats_n @ B_k + d_k)  with
    B_k = W0b + Wdec[:,3k:3k+3] @ W0c  and  d_k = b0 + bdec[3k:3k+3] @ W0c
    (host-precomputed).  u = x @ W0a is computed once per node tile and
    injected into each k's PSUM via an identity matmul.
  * Activations kept feature-major ([features, nodes]) on chip: matmul
    lhsT = weights (stationary), rhs = activations; per-partition bias.
    x is transposed on the host so no on-device transposes are needed.
  * The last layer flips to point-major (lhsT = activation tile) so the
    mandatory PSUM->SBUF relu pass writes the DMA-contiguous layout;
    its bias is injected as a K=1 matmul with a ones row.
  * PSUM accumulation chains serialize the PE array, so matmuls of
    different PSUM regions are interleaved to keep the pipeline deep.
  * cluster is input-independent: host numpy.
"""

import os
import sys

sys.path.insert(0, "/opt/trn_rl_repo")

import numpy as np

N_NODES = 50000
D_IN = 256
KPTS = 8
N_CORES = 8
R = N_NODES // N_CORES          # nodes per core
W = 1024                        # column (node) tile width
NTILES = (R + W - 1) // W       # 7 (6 full + 106 tail)

# compute dtype for matmul inputs: "float32" or "bfloat16"
CDT_NAME = os.environ.get("KERNEL_CDT", "bfloat16")

_cache = {}


def _ceil_div(a, b):
    return (a + b - 1) // b


def _build(cdt_name):
    from contextlib import ExitStack

    import concourse.mybir as mybir
    import concourse.tile as tile
    from concourse import bacc

    f32 = mybir.dt.float32
    cdt = getattr(mybir.dt, cdt_name)
    AF = mybir.ActivationFunctionType
    ALU = mybir.AluOpType

    nc = bacc.Bacc("TRN2", target_bir_lowering=False, debug=False,
                   enable_asserts=True, num_devices=N_CORES)

    def din(name, shape, dt):
        return nc.dram_tensor(name, shape, dt, kind="ExternalInput").ap()

    xT = din("xT", [D_IN, R], cdt)
    wblob = din("wblob", [128, 5400], cdt)
    fblob = din("fblob", [128, 24], f32)

    rel_out = nc.dram_tensor("rel", [R * KPTS, 3], f32, kind="ExternalOutput").ap()
    dec_out = nc.dram_tensor("dec", [R * KPTS, 256], f32, kind="ExternalOutput").ap()

    # node-major views: [R, K*3] / [R, K*256]
    rel_v = rel_out.rearrange("(n e) c -> n (e c)", e=KPTS)
    dec_v = dec_out.rearrange("(n e) c -> n (e c)", e=KPTS)

    with tile.TileContext(nc) as tc, ExitStack() as ctx:
        wts = ctx.enter_context(tc.tile_pool(name="wts", bufs=1))
        xt_p = ctx.enter_context(tc.tile_pool(name="xt", bufs=3))
        gact_p = ctx.enter_context(tc.tile_pool(name="gact", bufs=3))
        f_p = ctx.enter_context(tc.tile_pool(name="feats", bufs=2))
        u_p = ctx.enter_context(tc.tile_pool(name="u", bufs=2))
        h1_p = ctx.enter_context(tc.tile_pool(name="h1", bufs=3))
        h2_p = ctx.enter_context(tc.tile_pool(name="h2", bufs=3))
        d_p = ctx.enter_context(tc.tile_pool(name="dst", bufs=4))
        rel_p = ctx.enter_context(tc.tile_pool(name="relst", bufs=2))
        ps_mm = ctx.enter_context(tc.tile_pool(name="psmm", bufs=6, space="PSUM"))
        ps_fl = ctx.enter_context(tc.tile_pool(name="psfl", bufs=2, space="PSUM"))

        wb = wts.tile([128, 5400], cdt, tag="wblob")
        nc.sync.dma_start(wb[:, :], wblob[:, :])
        fb = wts.tile([128, 24], f32, tag="fblob")
        nc.sync.dma_start(fb[:, :], fblob[:, :])

        def wsl(off, cols, rows=128):
            return wb[:rows, off:off + cols]

        wg0_sb = [wsl(0, 256), wsl(256, 256)]
        wg1_sb = [wsl(512, 128), wsl(640, 128)]
        wg2_sb = wsl(768, 64)
        w0a_sb = [wsl(832, 256), wsl(1088, 256)]
        w1_sb = [wsl(1344, 256), wsl(1600, 256)]
        w2_sb = [wsl(1856, 256), wsl(2112, 256)]
        bmat_sb = wsl(2368, 2048)
        wdec_sb = wsl(4416, 24, 64)
        bdec_sb = wsl(4440, 192, 1)
        b2_sb = wsl(4632, 512, 1)
        on_sb = wsl(5272, 128, 1)
        bg0_sb = fb[:, 0:2]
        bg1_sb = fb[:, 2:3]
        bg2_sb = fb[:64, 3:4]
        dvec_sb = fb[:, 4:20]
        b1_sb = fb[:, 20:22]

        MMF = 512  # max matmul free size (fp32 PSUM bank)

        for t in range(NTILES):
            n0 = t * W
            w = min(W, R - n0)
            halves = [(ho, min(MMF, w - ho)) for ho in range(0, w, MMF)]
            nm = _ceil_div(w, 128)

            # ---- load xT tile ([256, w] as 2 partition chunks)
            xt_sb = []
            for c in range(2):
                xt = xt_p.tile([128, W], cdt, tag=f"xt{c}")
                nc.sync.dma_start(xt[:, :w], xT[c * 128:(c + 1) * 128, n0:n0 + w])
                xt_sb.append(xt)

            # ---- global MLP layer 0: 256 -> 256 (cin-outer: shared lhsT,
            #      adjacent matmuls hit different psum tiles)
            f0_sb = []
            for c2 in range(2):
                f0 = gact_p.tile([128, W], cdt, tag=f"f0{c2}")
                pss = [ps_mm.tile([128, MMF], f32, name=f"psg0_{t}_{c2}_{hi}",
                                  tag="psmm") for hi in range(len(halves))]
                for cin in range(2):
                    for hi, (ho, hw) in enumerate(halves):
                        nc.tensor.matmul(
                            pss[hi][:, :hw],
                            wg0_sb[cin][:, c2 * 128:(c2 + 1) * 128],
                            xt_sb[cin][:, ho:ho + hw],
                            start=(cin == 0), stop=(cin == 1))
                for hi, (ho, hw) in enumerate(halves):
                    nc.scalar.activation(f0[:, ho:ho + hw], pss[hi][:, :hw],
                                         AF.Relu, bias=bg0_sb[:, c2:c2 + 1])
                f0_sb.append(f0)

            # ---- layer 1: 256 -> 128
            f1 = gact_p.tile([128, W], cdt, tag="f1")
            pss = [ps_mm.tile([128, MMF], f32, name=f"psg1_{t}_{hi}", tag="psmm")
                   for hi in range(len(halves))]
            for cin in range(2):
                for hi, (ho, hw) in enumerate(halves):
                    nc.tensor.matmul(pss[hi][:, :hw], wg1_sb[cin],
                                     f0_sb[cin][:, ho:ho + hw],
                                     start=(cin == 0), stop=(cin == 1))
            for hi, (ho, hw) in enumerate(halves):
                nc.scalar.activation(f1[:, ho:ho + hw], pss[hi][:, :hw],
                                     AF.Relu, bias=bg1_sb[:, 0:1])

            # ---- layer 2: 128 -> 64  (feats)
            fT = f_p.tile([65, W], cdt, tag="fT")
            nc.gpsimd.memset(fT[64:65, :w], 1.0)
            for ho, hw in halves:
                ps = ps_mm.tile([128, MMF], f32, tag="psmm")
                nc.tensor.matmul(ps[:64, :hw], wg2_sb,
                                 f1[:, ho:ho + hw], start=True, stop=True)
                nc.scalar.activation(fT[:64, ho:ho + hw], ps[:64, :hw], AF.Relu,
                                     bias=bg2_sb[:64, 0:1])

            # ---- relative points: feats @ Wdec + bdec, point-major flip
            psr = ps_fl.tile([128, 512], f32, tag="psfl")
            nc.tensor.matmul(psr[:128, :nm * 24], on_sb[:1, :128],
                             bdec_sb[:1, :nm * 24], start=True, stop=False,
                             skip_group_check=True)
            for j in range(nm):
                mj = min(128, w - j * 128)
                nc.tensor.matmul(psr[:mj, j * 24:j * 24 + 24],
                                 fT[:64, j * 128:j * 128 + mj],
                                 wdec_sb[:64, :],
                                 start=False, stop=True, skip_group_check=True)
            relst = rel_p.tile([128, 192], f32, tag="relst")
            nc.vector.tensor_copy(relst[:, :nm * 24], psr[:, :nm * 24])
            if w == W:
                dram = rel_v[n0:n0 + w, :].rearrange("(j i) c -> i j c", i=128)
                nc.sync.dma_start(dram, relst[:, :].rearrange("i (j c) -> i j c", c=24))
            else:
                nc.sync.dma_start(rel_v[n0:n0 + w, :], relst[:w, :24])

            # ---- u = x @ W0a (shared across k), no activation
            u_sb = []
            for c2 in range(2):
                u = u_p.tile([128, W], cdt, tag=f"u{c2}")
                pss = [ps_mm.tile([128, MMF], f32, name=f"psu_{t}_{c2}_{hi}",
                                  tag="psmm") for hi in range(len(halves))]
                for cin in range(2):
                    for hi, (ho, hw) in enumerate(halves):
                        nc.tensor.matmul(
                            pss[hi][:, :hw],
                            w0a_sb[cin][:, c2 * 128:(c2 + 1) * 128],
                            xt_sb[cin][:, ho:ho + hw],
                            start=(cin == 0), stop=(cin == 1))
                for hi, (ho, hw) in enumerate(halves):
                    nc.vector.tensor_copy(u[:, ho:ho + hw], pss[hi][:, :hw])
                u_sb.append(u)

            # ---- per-k decode: h1 -> h2 -> h3 (flip) -> DMA
            for k in range(KPTS):
                # h1: inject u (identity matmul, 4 adjacent = shared lhsT),
                # then B_k accumulate, then fused DVE add-bias+relu
                h1_sb = [h1_p.tile([128, W], cdt, name=f"h1_{t}_{k}_{c2}",
                                   tag=f"h1{c2}") for c2 in range(2)]
                h1pre = [h1_p.tile([128, W], cdt, name=f"h1p_{t}_{k}_{c2}",
                                   tag=f"h1p{c2}") for c2 in range(2)]
                h1ps = {}
                for c2 in range(2):
                    for hi, (ho, hw) in enumerate(halves):
                        ps = ps_mm.tile([128, MMF], f32,
                                        name=f"psh1_{t}_{k}_{c2}_{hi}", tag="psmm")
                        h1ps[c2, hi] = ps
                        nc.tensor.matmul(
                            ps[:, :hw],
                            bmat_sb[:65, k * 256 + c2 * 128:k * 256 + (c2 + 1) * 128],
                            fT[:65, ho:ho + hw],
                            start=True, stop=True)
                for c2 in range(2):
                    for hi, (ho, hw) in enumerate(halves):
                        nc.vector.tensor_tensor(
                            out=h1pre[c2][:, ho:ho + hw], in0=h1ps[c2, hi][:, :hw],
                            in1=u_sb[c2][:, ho:ho + hw], op=ALU.add)
                        nc.vector.tensor_scalar_max(
                            out=h1_sb[c2][:, ho:ho + hw],
                            in0=h1pre[c2][:, ho:ho + hw], scalar1=0.0)

                # h2: 256 -> 256, cin-outer
                h2_sb = []
                for c2 in range(2):
                    h2 = h2_p.tile([128, W], cdt, tag=f"h2{c2}")
                    pss = [ps_mm.tile([128, MMF], f32,
                                      name=f"psh2_{t}_{k}_{c2}_{hi}", tag="psmm")
                           for hi in range(len(halves))]
                    for cin in range(2):
                        for hi, (ho, hw) in enumerate(halves):
                            nc.tensor.matmul(
                                pss[hi][:, :hw],
                                w1_sb[cin][:, c2 * 128:(c2 + 1) * 128],
                                h1_sb[cin][:, ho:ho + hw],
                                start=(cin == 0), stop=(cin == 1))
                    for hi, (ho, hw) in enumerate(halves):
                        nc.scalar.activation(h2[:, ho:ho + hw], pss[hi][:, :hw],
                                             AF.Relu, bias=b1_sb[:, c2:c2 + 1])
                    h2_sb.append(h2)

                # h3: point-major flip; pair-groups of 2 psum tiles with
                # matmuls interleaved across the 4 chunk-regions so no two
                # adjacent matmuls accumulate into the same region.
                dst = d_p.tile([128, 2048], f32, tag="dst")
                npairs = _ceil_div(nm, 2)
                for g in range(0, npairs, 2):
                    pairs = [p for p in (g, g + 1) if p < npairs]
                    pfs, meta = [], []
                    for p in pairs:
                        pf = ps_fl.tile([128, 512], f32,
                                        name=f"pf_{t}_{k}_{p}", tag="psfl")
                        chunks = [j for j in (p * 2, p * 2 + 1) if j < nm]
                        mjs = [min(128, w - j * 128) for j in chunks]
                        pw = 256 * len(chunks)
                        nc.tensor.matmul(pf[:max(mjs), :pw], on_sb[:1, :max(mjs)],
                                         b2_sb[:1, :pw], start=True, stop=False,
                                         skip_group_check=True)
                        pfs.append(pf)
                        meta.append((chunks, mjs, pw))
                    for cin in range(2):
                        for jj in range(2):
                            for pi in range(len(pairs)):
                                chunks, mjs, pw = meta[pi]
                                if jj >= len(chunks):
                                    continue
                                j, mj = chunks[jj], mjs[jj]
                                off = jj * 256
                                nc.tensor.matmul(
                                    pfs[pi][:mj, off:off + 256],
                                    h2_sb[cin][:, j * 128:j * 128 + mj],
                                    w2_sb[cin], start=False, stop=(cin == 1),
                                    skip_group_check=True)
                    for pi, p in enumerate(pairs):
                        chunks, mjs, pw = meta[pi]
                        o = dst[:, p * 512:p * 512 + pw]
                        if p % 2 == 0:
                            nc.scalar.activation(o, pfs[pi][:, :pw], AF.Relu,
                                                 bias=0.0)
                        else:
                            nc.vector.tensor_scalar_max(out=o, in0=pfs[pi][:, :pw],
                                                        scalar1=0.0)
                if w == W:
                    dram = dec_v[n0:n0 + w, k * 256:(k + 1) * 256].rearrange(
                        "(j i) c -> i j c", i=128)
                    nc.sync.dma_start(
                        dram, dst[:, :].rearrange("i (j c) -> i j c", c=256))
                else:
                    nc.sync.dma_start(dec_v[n0:n0 + w, k * 256:(k + 1) * 256],
                                      dst[:w, :256])

    nc.compile()
    return nc


def _get_nc(cdt_name):
    if cdt_name not in _cache:
        _cache[cdt_name] = _build(cdt_name)
    return _cache[cdt_name]


def _prep_inputs(inputs, cdt_name):
    import ml_dtypes

    np_cdt = np.float32 if cdt_name == "float32" else ml_dtypes.bfloat16

    def c(a):
        return np.ascontiguousarray(a, dtype=np_cdt)

    x = np.asarray(inputs["input_features"], dtype=np.float32)
    W0 = np.asarray(inputs["W0"], dtype=np.float32)
    W0a, W0b, W0c = W0[:256], W0[256:320], W0[320:323]
    Wdec = np.asarray(inputs["Wdec"], dtype=np.float32)
    bdec = np.asarray(inputs["bdec"], dtype=np.float32)
    b0 = np.asarray(inputs["b0"], dtype=np.float32)

    B = np.concatenate(
        [np.vstack([W0b + Wdec[:, 3 * k:3 * k + 3] @ W0c,
                    (b0 + bdec[3 * k:3 * k + 3] @ W0c)[None, :]])
         for k in range(KPTS)], axis=1)
    D = np.stack([b0 + bdec[3 * k:3 * k + 3] @ W0c for k in range(KPTS)])

    def pcol(v):  # [n*128] -> [128, n] per-partition bias columns
        return np.ascontiguousarray(v.reshape(-1, 128).T, dtype=np.float32)

    wb = np.zeros((128, 5400), dtype=np.float32)

    def put(off, arr, rows=None):
        a = np.asarray(arr, np.float32)
        if a.ndim == 1:
            a = a[None, :]
        r, cols = a.shape
        wb[:r, off:off + cols] = a

    Wg0 = np.asarray(inputs["Wg0"], np.float32)
    Wg1 = np.asarray(inputs["Wg1"], np.float32)
    Wg2 = np.asarray(inputs["Wg2"], np.float32)
    W1 = np.asarray(inputs["W1"], np.float32)
    W2 = np.asarray(inputs["W2"], np.float32)
    b2 = np.asarray(inputs["b2"], np.float32)
    put(0, Wg0[0:128]); put(256, Wg0[128:256])
    put(512, Wg1[0:128]); put(640, Wg1[128:256])
    put(768, Wg2)
    put(832, W0a[0:128]); put(1088, W0a[128:256])
    put(1344, W1[0:128]); put(1600, W1[128:256])
    put(1856, W2[0:128]); put(2112, W2[128:256])
    put(2368, B)
    put(4416, Wdec)
    put(4440, np.tile(bdec, KPTS))
    put(4632, np.tile(b2, 2))
    put(5144, np.eye(128, dtype=np.float32))
    put(5272, np.ones(128, dtype=np.float32))

    fbv = np.zeros((128, 24), dtype=np.float32)

    def pcol(v):  # [n*128] -> [128, n] per-partition bias columns
        return np.ascontiguousarray(np.asarray(v, np.float32).reshape(-1, 128).T)

    fbv[:, 0:2] = pcol(inputs["bg0"])
    fbv[:, 2:3] = pcol(inputs["bg1"])
    fbv[:64, 3:4] = np.asarray(inputs["bg2"], np.float32).reshape(64, 1)
    fbv[:, 4:20] = D.reshape(KPTS, 2, 128).transpose(2, 0, 1).reshape(128, 16)
    fbv[:, 20:22] = pcol(inputs["b1"])

    shared = {"wblob": c(wb), "fblob": fbv}
    in_maps = []
    for cid in range(N_CORES):
        m = dict(shared)
        m["xT"] = c(x[cid * R:(cid + 1) * R].T)
        in_maps.append(m)
    return in_maps


def _install_ntff_hook_module():
    """Make `antenv.axon_hooks` importable (image ships without it)."""
    import importlib.util

    try:
        import antenv.axon_hooks  # noqa: F401
        return
    except ImportError:
        pass
    path = "/opt/trn_rl_repo/antenv/axon_hooks.py"
    if not os.path.exists(path):
        return
    spec = importlib.util.spec_from_file_location("antenv.axon_hooks", path)
    mod = importlib.util.module_from_spec(spec)
    spec.loader.exec_module(mod)
    sys.modules["antenv.axon_hooks"] = mod
    import antenv

    antenv.axon_hooks = mod


def _maybe_enable_ldw_opt():
    """Optionally flip walrus --enable-ldw-opt via a driver shim
    (KERNEL_LDW_OPT=1). Correctness is validated against the reference."""
    if os.environ.get("KERNEL_LDW_OPT") != "1":
        return
    from concourse import bass_utils as bu

    if getattr(bu, "_ldw_shim", None):
        return
    real = bu.get_walrus_driver()
    shim = "/tmp/walrus_ldw_shim.sh"
    with open(shim, "w") as f:
        f.write("#!/bin/sh\n")
        f.write('args=""\n')
        f.write('for a in "$@"; do\n')
        f.write('  case "$a" in --enable-ldw-opt=false) a="--enable-ldw-opt=true";; esac\n')
        f.write('  args="$args \"$a\""\n')
        f.write("done\n")
        f.write(f'eval exec "{real}" $args\n')
    os.chmod(shim, 0o755)
    bu.get_walrus_driver = lambda: shim
    bu._ldw_shim = True


def kernel(**inputs):
    from concourse.bass_utils import run_bass_kernel_spmd

    _maybe_enable_ldw_opt()
    nc = _get_nc(CDT_NAME)
    in_maps = _prep_inputs(inputs, CDT_NAME)

    trace = os.environ.get("BASS_KERNEL_TRACE") == "1"
    if trace:
        _install_ntff_hook_module()
    res = run_bass_kernel_spmd(nc, in_maps, core_ids=list(range(N_CORES)),
                               trace=trace)
    global LAST_RESULT
    LAST_RESULT = res

    rel = np.concatenate([res.results[c]["rel"] for c in range(N_CORES)], axis=0)
    dec = np.concatenate([res.results[c]["dec"] for c in range(N_CORES)], axis=0)
    cluster = np.repeat(np.arange(N_NODES, dtype=np.int32), KPTS)
    return rel, dec, cluster
